# revision 5
# baseline (speedup 1.0000x reference)
"""Trainium2 Bass kernel for nn_DataPreprocessor: row-interleave + 16x16 patch
extraction, implemented as a pure data-movement (permutation) kernel.

Reference semantics (per sample):
  data: [2, 65536] -> R: [256, 512] with R[2k]=data[0].reshape(128,512)[k],
  R[2k+1]=data[1].reshape(128,512)[k] -> non-overlapping 16x16 patches,
  row-major, each flattened -> out: [512, 256].

Index algebra (per sample), with z1 in [0,16), z2 in [0,32), ph in [0,8),
e in [0,2), q in [0,16):
  out[z1*32+z2, (2*ph+e)*16+q] = data[e, z1*4096 + ph*512 + z2*16 + q]
So with out flat = z1*8192 + z2*256 + ph*32 + e*16 + q the kernel is the pure
5D transpose (e, z1, ph, z2, q) -> (z1, z2, ph, e, q).

Strategy: batch-shard 256 samples over 8 cores (32/core). Per tile of 8
samples, SBUF partition p = (b_local, z1) [128 partitions]:
  - 2 load DMAs (one per e) on the SP HWDGE ring; HBM side reads 16KB
    contiguous runs
  - 2 DVE copies permute the free dim (e,ph,z2,q)->(z2,ph,e,q) on-chip
  - 1 store DMA on the ACT HWDGE ring; HBM side is one contiguous 4MB block
Raw bass (not Tile): walrus's DMA_DIRECT2D struct only admits one sync-wait
command per DMA, so all cross-engine ordering is done with standalone
wait_ge instructions and a 3-semaphore ring pipeline.
"""

import sys

for _p in ("/opt/trn_rl_repo",):
    if _p not in sys.path:
        sys.path.insert(0, _p)

import numpy as np

import concourse.bass as bass
import concourse.mybir as mybir
from concourse.bass_utils import run_bass_kernel_spmd

N_CORES = 8
B = 256
B_PER_CORE = B // N_CORES          # 32
SAMPLES_PER_TILE = 8               # 8 samples x 16 z1 = 128 partitions
Z1, Z2, PH, E, QQ = 16, 32, 8, 2, 16
FREE = E * PH * Z2 * QQ            # 8192 elements = 32KB per partition
NPART = SAMPLES_PER_TILE * Z1      # 128


def build_nc(b_per_core: int = B_PER_CORE) -> bass.Bass:
    n_tiles = b_per_core // SAMPLES_PER_TILE
    f32 = mybir.dt.float32

    nc = bass.Bass()
    x = nc.dram_tensor("x", [b_per_core, 2, 65536], f32, kind="ExternalInput")
    y = nc.dram_tensor("y", [b_per_core, Z1 * Z2, PH * E * QQ], f32,
                       kind="ExternalOutput")

    # [b, e, z1, r] view of the input; r = ph*512 + z2*16 + q is a 16KB run
    xv = x.rearrange("b e (z1 r) -> b e z1 r", z1=Z1)

    with (
        nc.sbuf_tensor([NPART, FREE], f32) as tin0,
        nc.sbuf_tensor([NPART, FREE], f32) as tin1,
        nc.sbuf_tensor([NPART, FREE], f32) as tout0,
        nc.sbuf_tensor([NPART, FREE], f32) as tout1,
        nc.semaphore("ld_sem0") as ld_sem0,
        nc.semaphore("ld_sem1") as ld_sem1,
        nc.semaphore("st_sem0") as st_sem0,
        nc.semaphore("st_sem1") as st_sem1,
        nc.semaphore("cp_sem") as cp_sem,
        nc.Block() as block,
    ):
        tins = [tin0, tin1]
        touts = [tout0, tout1]
        ld_sems = [ld_sem0, ld_sem1]
        st_sems = [st_sem0, st_sem1]
        # DMA-completion sems arrive as 16 independent +1s per DMA, and
        # several DMAs can be in flight at once; a threshold is only
        # unambiguous if every DMA that ever increments the sem up to that
        # point is covered by it. Per-slot sems guarantee that: slot t%2's
        # sem only sees tiles t, t-2, ... and tile t+2 cannot issue until
        # the wait for tile t has already cleared (gated through cp_sem).

        @block.sync
        def _(sync):
            for t in range(n_tiles):
                tin = tins[t % 2]
                if t >= 2:
                    # WAR: copies of tile t-2 must have drained tin slot
                    sync.wait_ge(cp_sem, 2 * (t - 2) + 2)
                b0 = t * SAMPLES_PER_TILE
                for e in range(E):
                    sync.dma_start(
                        out=tin[:, e * 4096:(e + 1) * 4096],
                        in_=xv[b0:b0 + SAMPLES_PER_TILE, e],
                    ).then_inc(ld_sems[t % 2], 16)

        @block.scalar
        def _(scalar):
            for t in range(n_tiles):
                tout = touts[t % 2]
                # RAW: both shuffle copies of tile t done
                scalar.wait_ge(cp_sem, 2 * t + 2)
                b0 = t * SAMPLES_PER_TILE
                scalar.dma_start(
                    out=y[b0:b0 + SAMPLES_PER_TILE],
                    in_=tout[:],
                ).then_inc(st_sems[t % 2], 16)

        @block.vector
        def _(vector):
            for t in range(n_tiles):
                tin = tins[t % 2]
                tout = touts[t % 2]
                # RAW: both loads of tile t landed
                vector.wait_ge(ld_sems[t % 2], 32 * (t // 2 + 1))
                if t >= 2:
                    # WAR: store of tile t-2 must have drained tout slot
                    vector.wait_ge(st_sems[t % 2], 16 * (t // 2))
                tin_v = tin.rearrange("p (e ph z2 q) -> p e ph z2 q",
                                      e=E, ph=PH, z2=Z2, q=QQ)
                tout_v = tout.rearrange("p (z2 ph e q) -> p e ph z2 q",
                                        z2=Z2, ph=PH, e=E, q=QQ)
                for e in range(E):
                    vector.tensor_copy(tout_v[:, e], tin_v[:, e]).then_inc(
                        cp_sem, 1)

    return nc


_NC_CACHE: dict = {}


def _get_nc():
    if "nc" not in _NC_CACHE:
        _NC_CACHE["nc"] = build_nc()
    return _NC_CACHE["nc"]


def kernel(data: np.ndarray, _trace: bool = False):
    data = np.ascontiguousarray(data, dtype=np.float32)
    assert data.shape == (B, 2, 65536), data.shape
    nc = _get_nc()
    in_maps = [{"x": data[i * B_PER_CORE:(i + 1) * B_PER_CORE]}
               for i in range(N_CORES)]
    res = run_bass_kernel_spmd(nc, in_maps, list(range(N_CORES)),
                               trace=_trace)
    out = np.concatenate([res.results[i]["y"] for i in range(N_CORES)], axis=0)
    if _trace:
        return out, res
    return out


# revision 8
# speedup vs baseline: 1.5505x; 1.5505x over previous
"""Trainium2 Bass kernel for nn_DataPreprocessor: row-interleave + 16x16 patch
extraction, implemented as a pure data-movement (permutation) kernel.

Reference semantics (per sample):
  data: [2, 65536] -> R: [256, 512] with R[2k]=data[0].reshape(128,512)[k],
  R[2k+1]=data[1].reshape(128,512)[k] -> non-overlapping 16x16 patches,
  row-major, each flattened -> out: [512, 256].

Index algebra (per sample), with z1 in [0,16), z2 in [0,32), ph in [0,8),
e in [0,2), q in [0,16):
  out[z1*32+z2, (2*ph+e)*16+q] = data[e, z1*4096 + ph*512 + z2*16 + q]
So with out flat = z1*8192 + z2*256 + ph*32 + e*16 + q the kernel is the pure
5D transpose (e, z1, ph, z2, q) -> (z1, z2, ph, e, q).

Strategy: batch-shard 256 samples over 8 cores (32/core). Per tile of 8
samples, SBUF partition p = (b_local, z1) [128 partitions]:
  - 2 load DMAs (one per e) on the SP HWDGE ring; HBM side reads 16KB
    contiguous runs
  - 2 DVE copies permute the free dim (e,ph,z2,q)->(z2,ph,e,q) on-chip
  - 1 store DMA on the ACT HWDGE ring; HBM side is one contiguous 4MB block
Raw bass (not Tile): walrus's DMA_DIRECT2D struct only admits one sync-wait
command per DMA, so all cross-engine ordering is done with standalone
wait_ge instructions and a 3-semaphore ring pipeline.
"""

import sys

for _p in ("/opt/trn_rl_repo",):
    if _p not in sys.path:
        sys.path.insert(0, _p)

import numpy as np

import concourse.bass as bass
import concourse.mybir as mybir
from concourse.bass_utils import run_bass_kernel_spmd

N_CORES = 8
B = 256
B_PER_CORE = B // N_CORES          # 32
SAMPLES_PER_TILE = 8               # 8 samples x 16 z1 = 128 partitions
Z1, Z2, PH, E, QQ = 16, 32, 8, 2, 16
FREE = E * PH * Z2 * QQ            # 8192 elements = 32KB per partition
NPART = SAMPLES_PER_TILE * Z1      # 128


def build_nc(b_per_core: int = B_PER_CORE) -> bass.Bass:
    n_tiles = b_per_core // SAMPLES_PER_TILE
    f32 = mybir.dt.float32

    nc = bass.Bass()
    x = nc.dram_tensor("x", [b_per_core, 2, 65536], f32, kind="ExternalInput")
    y = nc.dram_tensor("y", [b_per_core, Z1 * Z2, PH * E * QQ], f32,
                       kind="ExternalOutput")

    # SBUF partition p = z1*8 + b_local (z1 outermost): the HWDGE spreads a
    # DMA over the 16 SDMA engines by the HBM AP's outer dim index, so the
    # outer dim must be >= 16 (z1 is 16; b_local is only 8).
    # [z1, b, e, r] view of the input; r = ph*512 + z2*16 + q is a 16KB run
    xv = x.rearrange("b e (z1 r) -> z1 b e r", z1=Z1)
    # [z1, b, f] view of the output; f = z2*256 + ph*32 + e*16 + q
    yv = y.rearrange("b (z1 f2) c -> z1 b (f2 c)", z1=Z1)

    with (
        nc.sbuf_tensor([NPART, FREE], f32) as tin0,
        nc.sbuf_tensor([NPART, FREE], f32) as tin1,
        nc.sbuf_tensor([NPART, FREE], f32) as tout0,
        nc.sbuf_tensor([NPART, FREE], f32) as tout1,
        nc.semaphore("ld_sem0") as ld_sem0,
        nc.semaphore("ld_sem1") as ld_sem1,
        nc.semaphore("st_sem0") as st_sem0,
        nc.semaphore("st_sem1") as st_sem1,
        nc.semaphore("cp_sem") as cp_sem,
        nc.Block() as block,
    ):
        tins = [tin0, tin1]
        touts = [tout0, tout1]
        ld_sems = [ld_sem0, ld_sem1]
        st_sems = [st_sem0, st_sem1]
        # DMA-completion sems arrive as 16 independent +1s per DMA, and
        # several DMAs can be in flight at once; a threshold is only
        # unambiguous if every DMA that ever increments the sem up to that
        # point is covered by it. Per-slot sems guarantee that: slot t%2's
        # sem only sees tiles t, t-2, ... and tile t+2 cannot issue until
        # the wait for tile t has already cleared (gated through cp_sem).

        @block.sync
        def _(sync):
            for t in range(n_tiles):
                tin = tins[t % 2]
                if t >= 2:
                    # WAR: copies of tile t-2 must have drained tin slot
                    sync.wait_ge(cp_sem, 2 * (t - 2) + 2)
                b0 = t * SAMPLES_PER_TILE
                for e in range(E):
                    sync.dma_start(
                        out=tin[:, e * 4096:(e + 1) * 4096],
                        in_=xv[:, b0:b0 + SAMPLES_PER_TILE, e],
                    ).then_inc(ld_sems[t % 2], 16)

        @block.scalar
        def _(scalar):
            for t in range(n_tiles):
                tout = touts[t % 2]
                # RAW: both shuffle copies of tile t done
                scalar.wait_ge(cp_sem, 2 * t + 2)
                b0 = t * SAMPLES_PER_TILE
                scalar.dma_start(
                    out=yv[:, b0:b0 + SAMPLES_PER_TILE],
                    in_=tout[:],
                ).then_inc(st_sems[t % 2], 16)

        @block.vector
        def _(vector):
            for t in range(n_tiles):
                tin = tins[t % 2]
                tout = touts[t % 2]
                # RAW: both loads of tile t landed
                vector.wait_ge(ld_sems[t % 2], 32 * (t // 2 + 1))
                if t >= 2:
                    # WAR: store of tile t-2 must have drained tout slot
                    vector.wait_ge(st_sems[t % 2], 16 * (t // 2))
                tin_v = tin.rearrange("p (e ph z2 q) -> p e ph z2 q",
                                      e=E, ph=PH, z2=Z2, q=QQ)
                tout_v = tout.rearrange("p (z2 ph e q) -> p e ph z2 q",
                                        z2=Z2, ph=PH, e=E, q=QQ)
                for e in range(E):
                    vector.tensor_copy(tout_v[:, e], tin_v[:, e]).then_inc(
                        cp_sem, 1)

    return nc


_NC_CACHE: dict = {}


def _get_nc():
    if "nc" not in _NC_CACHE:
        _NC_CACHE["nc"] = build_nc()
    return _NC_CACHE["nc"]


def kernel(data: np.ndarray, _trace: bool = False):
    data = np.ascontiguousarray(data, dtype=np.float32)
    assert data.shape == (B, 2, 65536), data.shape
    nc = _get_nc()
    in_maps = [{"x": data[i * B_PER_CORE:(i + 1) * B_PER_CORE]}
               for i in range(N_CORES)]
    res = run_bass_kernel_spmd(nc, in_maps, list(range(N_CORES)),
                               trace=_trace)
    out = np.concatenate([res.results[i]["y"] for i in range(N_CORES)], axis=0)
    if _trace:
        return out, res
    return out


# revision 9
# speedup vs baseline: 1.6218x; 1.0459x over previous
"""Trainium2 Bass kernel for nn_DataPreprocessor: row-interleave + 16x16 patch
extraction, implemented as a pure data-movement (permutation) kernel.

Reference semantics (per sample):
  data: [2, 65536] -> R: [256, 512] with R[2k]=data[0].reshape(128,512)[k],
  R[2k+1]=data[1].reshape(128,512)[k] -> non-overlapping 16x16 patches,
  row-major, each flattened -> out: [512, 256].

Index algebra (per sample), with z1 in [0,16), z2 in [0,32), ph in [0,8),
e in [0,2), q in [0,16):
  out[z1*32+z2, (2*ph+e)*16+q] = data[e, z1*4096 + ph*512 + z2*16 + q]
With out flat = z1*8192 + z2*256 + ph*32 + e*16 + q the kernel is the pure
5D transpose (e, z1, ph, z2, q) -> (z1, z2, ph, e, q).

Strategy: batch-shard 256 samples over 8 cores (32/core), processed as 2
tiles of 16 samples. Split z1 = z1h*2 + z1l; SBUF partition p = b*8 + z1h
(b in [0,16) local). Then:
  - loads (one per e): HBM AP [b:16][z1h:8][(z1l r):8192] -- 32KB contiguous
    descriptors, outer dim 16 so the HWDGE spreads over all 16 SDMA engines
  - shuffle: 4 DVE copies per tile, (e,z1l)-indexed, permuting the free dim
    (ph,z2,q) -> (z2,ph,q) blocks into out order within each partition
  - stores (one per z1l): HBM AP [b:16][z1h:8][8192] -- 32KB descriptors,
    outer 16
All DMA descriptors are the full-rate 32KB shape (the 181us -> 117us -> now
journey: never give HWDGE an AP whose outer dim is <16, and never let
descriptors shrink below 32KB when avoidable).

Raw bass (not Tile): walrus's DMA_DIRECT2D struct admits only one sync-wait
command per DMA, so cross-engine ordering uses standalone wait_ge
instructions. DMA-completion semaphores arrive as 16 independent +1s per
DMA, so each wait threshold must only ever count DMAs covered by it:
dedicated sems per (tile, e) for loads and per tout-slot for stores.
"""

import sys

for _p in ("/opt/trn_rl_repo",):
    if _p not in sys.path:
        sys.path.insert(0, _p)

import numpy as np

import concourse.bass as bass
import concourse.mybir as mybir
from concourse.bass_utils import run_bass_kernel_spmd

N_CORES = 8
B = 256
B_PER_CORE = B // N_CORES          # 32
SAMPLES_PER_TILE = 16              # 16 samples x 8 z1h = 128 partitions
Z1H, Z1L, Z2, PH, E, QQ = 8, 2, 32, 8, 2, 16
FREE_IN = E * Z1L * PH * Z2 * QQ   # 16384 elements = 64KB per partition
FREE_OUT = PH * Z2 * E * QQ        # 8192 elements = 32KB per partition
NPART = 128


def build_nc(b_per_core: int = B_PER_CORE) -> bass.Bass:
    n_tiles = b_per_core // SAMPLES_PER_TILE
    f32 = mybir.dt.float32

    nc = bass.Bass()
    x = nc.dram_tensor("x", [b_per_core, 2, 65536], f32, kind="ExternalInput")
    y = nc.dram_tensor("y", [b_per_core, 512, 256], f32,
                       kind="ExternalOutput")

    # load view: [b, z1h, e, (z1l r)] ; (z1l r) is a 32KB contiguous run
    xv = x.rearrange("b e (z1h zr) -> b z1h e zr", z1h=Z1H)
    # store view: [b, z1h, z1l, (z2 c)] ; (z2 c) is a 32KB contiguous run
    yv = y.rearrange("b (z1h z1l z2) c -> b z1h z1l (z2 c)",
                     z1h=Z1H, z1l=Z1L)

    with (
        nc.sbuf_tensor([NPART, FREE_IN], f32) as tin0,
        nc.sbuf_tensor([NPART, FREE_IN], f32) as tin1,
        nc.sbuf_tensor([NPART, FREE_OUT], f32) as tout0,
        nc.sbuf_tensor([NPART, FREE_OUT], f32) as tout1,
        nc.semaphore("ld00") as ld00,
        nc.semaphore("ld01") as ld01,
        nc.semaphore("ld10") as ld10,
        nc.semaphore("ld11") as ld11,
        nc.semaphore("st0") as st0,
        nc.semaphore("st1") as st1,
        nc.semaphore("cp_sem") as cp_sem,
        nc.Block() as block,
    ):
        tins = [tin0, tin1]
        touts = [tout0, tout1]
        ld_sems = [[ld00, ld01], [ld10, ld11]]
        st_sems = [st0, st1]

        @block.sync
        def _(sync):
            # loads stream back-to-back with no waits: each tile has its
            # own tin buffer, so there is no SBUF reuse hazard on loads
            for t in range(n_tiles):
                b0 = t * SAMPLES_PER_TILE
                for e in range(E):
                    sync.dma_start(
                        out=tins[t][:, e * 8192:(e + 1) * 8192],
                        in_=xv[b0:b0 + SAMPLES_PER_TILE, :, e],
                    ).then_inc(ld_sems[t][e], 16)

        @block.vector
        def _(vector):
            for t in range(n_tiles):
                tin = tins[t]
                for z1l in range(Z1L):
                    s = (t * Z1L + z1l) % 2
                    tout = touts[s]
                    if t * Z1L + z1l >= 2:
                        # WAR: the store that last read this tout slot
                        vector.wait_ge(st_sems[s], 16 * ((t * Z1L + z1l) // 2))
                    for e in range(E):
                        vector.wait_ge(ld_sems[t][e], 16)
                        # src: f = e*8192 + z1l*4096 + ph*512 + z2*16 + q
                        src = tin.rearrange(
                            "p (e z1l ph z2 q) -> p e z1l ph z2 q",
                            e=E, z1l=Z1L, ph=PH, z2=Z2, q=QQ)[:, e, z1l]
                        # dst: f' = z2*256 + ph*32 + e*16 + q
                        dst = tout.rearrange(
                            "p (z2 ph e q) -> p e ph z2 q",
                            z2=Z2, ph=PH, e=E, q=QQ)[:, e]
                        vector.tensor_copy(dst, src).then_inc(cp_sem, 1)

        @block.scalar
        def _(scalar):
            for t in range(n_tiles):
                b0 = t * SAMPLES_PER_TILE
                for z1l in range(Z1L):
                    s = (t * Z1L + z1l) % 2
                    # RAW: both copies (e=0,1) for this (t, z1l) done
                    scalar.wait_ge(cp_sem, 4 * t + 2 * z1l + 2)
                    scalar.dma_start(
                        out=yv[b0:b0 + SAMPLES_PER_TILE, :, z1l],
                        in_=touts[s][:],
                    ).then_inc(st_sems[s], 16)

    return nc


_NC_CACHE: dict = {}


def _get_nc():
    if "nc" not in _NC_CACHE:
        _NC_CACHE["nc"] = build_nc()
    return _NC_CACHE["nc"]


def kernel(data: np.ndarray, _trace: bool = False):
    data = np.ascontiguousarray(data, dtype=np.float32)
    assert data.shape == (B, 2, 65536), data.shape
    nc = _get_nc()
    in_maps = [{"x": data[i * B_PER_CORE:(i + 1) * B_PER_CORE]}
               for i in range(N_CORES)]
    res = run_bass_kernel_spmd(nc, in_maps, list(range(N_CORES)),
                               trace=_trace)
    out = np.concatenate([res.results[i]["y"] for i in range(N_CORES)], axis=0)
    if _trace:
        return out, res
    return out


# revision 12
# speedup vs baseline: 1.6633x; 1.0256x over previous
"""Trainium2 Bass kernel for nn_DataPreprocessor: row-interleave + 16x16 patch
extraction, implemented as a pure data-movement (permutation) kernel.

Reference semantics (per sample):
  data: [2, 65536] -> R: [256, 512] with R[2k]=data[0].reshape(128,512)[k],
  R[2k+1]=data[1].reshape(128,512)[k] -> non-overlapping 16x16 patches,
  row-major, each flattened -> out: [512, 256].

Index algebra (per sample), with z1 in [0,16), z2 in [0,32), ph in [0,8),
e in [0,2), q in [0,16):
  out[z1*32+z2, (2*ph+e)*16+q] = data[e, z1*4096 + ph*512 + z2*16 + q]
With out flat = z1*8192 + z2*256 + ph*32 + e*16 + q the kernel is the pure
5D transpose (e, z1, ph, z2, q) -> (z1, z2, ph, e, q).

Strategy: batch-shard 256 samples over 8 cores (32/core), processed as 2
tiles of 16 samples. Split z1 = z1h*2 + z1l; SBUF partition p = b*8 + z1h
(b in [0,16) local). Then:
  - loads (one per e): HBM AP [b:16][z1h:8][(z1l r):8192] -- 32KB contiguous
    descriptors, outer dim 16 so the HWDGE spreads over all 16 SDMA engines
  - shuffle: 4 DVE copies per tile, (e,z1l)-indexed, permuting the free dim
    (ph,z2,q) -> (z2,ph,q) blocks into out order within each partition
  - stores (one per z1l): HBM AP [b:16][z1h:8][8192] -- 32KB descriptors,
    outer 16
All DMA descriptors are the full-rate 32KB shape (the 181us -> 117us -> now
journey: never give HWDGE an AP whose outer dim is <16, and never let
descriptors shrink below 32KB when avoidable).

Raw bass (not Tile): walrus's DMA_DIRECT2D struct admits only one sync-wait
command per DMA, so cross-engine ordering uses standalone wait_ge
instructions. DMA-completion semaphores arrive as 16 independent +1s per
DMA, so each wait threshold must only ever count DMAs covered by it:
dedicated sems per (tile, e) for loads and per tout-slot for stores.
"""

import sys

for _p in ("/opt/trn_rl_repo",):
    if _p not in sys.path:
        sys.path.insert(0, _p)

import numpy as np

import concourse.bass as bass
import concourse.mybir as mybir
from concourse.bass_utils import run_bass_kernel_spmd

N_CORES = 8
B = 256
B_PER_CORE = B // N_CORES          # 32
SAMPLES_PER_TILE = 16              # 16 samples x 8 z1h = 128 partitions
Z1H, Z1L, Z2, PH, E, QQ = 8, 2, 32, 8, 2, 16
FREE_IN = E * Z1L * PH * Z2 * QQ   # 16384 elements = 64KB per partition
FREE_OUT = PH * Z2 * E * QQ        # 8192 elements = 32KB per partition
NPART = 128


def build_nc(b_per_core: int = B_PER_CORE) -> bass.Bass:
    n_tiles = b_per_core // SAMPLES_PER_TILE
    f32 = mybir.dt.float32

    nc = bass.Bass()
    x = nc.dram_tensor("x", [b_per_core, 2, 65536], f32, kind="ExternalInput")
    y = nc.dram_tensor("y", [b_per_core, 512, 256], f32,
                       kind="ExternalOutput")

    # load view: [b, z1h, e, z1l, r] ; r is a 16KB contiguous run
    xv = x.rearrange("b e (z1h z1l r) -> b z1h e z1l r", z1h=Z1H, z1l=Z1L)
    # store view: [b, z1h, z1l, (z2 c)] ; (z2 c) is a 32KB contiguous run
    yv = y.rearrange("b (z1h z1l z2) c -> b z1h z1l (z2 c)",
                     z1h=Z1H, z1l=Z1L)

    with (
        nc.sbuf_tensor([NPART, FREE_IN], f32) as tin0,
        nc.sbuf_tensor([NPART, FREE_IN], f32) as tin1,
        nc.sbuf_tensor([NPART, FREE_OUT], f32) as tout0,
        nc.sbuf_tensor([NPART, FREE_OUT], f32) as tout1,
        nc.semaphore("ld00") as ld00,
        nc.semaphore("ld01") as ld01,
        nc.semaphore("ld10") as ld10,
        nc.semaphore("ld11") as ld11,
        nc.semaphore("st0") as st0,
        nc.semaphore("st1") as st1,
        nc.semaphore("cp_sem") as cp_sem,
        nc.Block() as block,
    ):
        tins = [tin0, tin1]
        touts = [tout0, tout1]
        ld_sems = [[ld00, ld01], [ld10, ld11]]
        st_sems = [st0, st1]

        @block.sync
        def _(sync):
            # loads stream back-to-back with no waits: each tile has its
            # own tin buffer, so there is no SBUF reuse hazard on loads.
            # One DMA per (e, z1l) quarter: 16KB descriptors -- measured
            # faster on the HBM read side than 32KB descriptors that comb
            # over the e-interleave at 50% duty.
            for t in range(n_tiles):
                b0 = t * SAMPLES_PER_TILE
                for e in range(E):
                    for z1l in range(Z1L):
                        off = e * 8192 + z1l * 4096
                        sync.dma_start(
                            out=tins[t][:, off:off + 4096],
                            in_=xv[b0:b0 + SAMPLES_PER_TILE, :, e, z1l],
                        ).then_inc(ld_sems[t][e], 16)

        @block.vector
        def _(vector):
            for t in range(n_tiles):
                tin = tins[t]
                for z1l in range(Z1L):
                    s = (t * Z1L + z1l) % 2
                    tout = touts[s]
                    if t * Z1L + z1l >= 2:
                        # WAR: the store that last read this tout slot
                        vector.wait_ge(st_sems[s], 16 * ((t * Z1L + z1l) // 2))
                    for e in range(E):
                        # both z1l-quarter loads of this (tile, e)
                        vector.wait_ge(ld_sems[t][e], 32)
                        # src: f = e*8192 + z1l*4096 + ph*512 + z2*16 + q
                        src = tin.rearrange(
                            "p (e z1l ph z2 q) -> p e z1l ph z2 q",
                            e=E, z1l=Z1L, ph=PH, z2=Z2, q=QQ)[:, e, z1l]
                        # dst: f' = z2*256 + ph*32 + e*16 + q
                        dst = tout.rearrange(
                            "p (z2 ph e q) -> p e ph z2 q",
                            z2=Z2, ph=PH, e=E, q=QQ)[:, e]
                        vector.tensor_copy(dst, src).then_inc(cp_sem, 1)

        @block.scalar
        def _(scalar):
            for t in range(n_tiles):
                b0 = t * SAMPLES_PER_TILE
                for z1l in range(Z1L):
                    s = (t * Z1L + z1l) % 2
                    # RAW: both copies (e=0,1) for this (t, z1l) done
                    scalar.wait_ge(cp_sem, 4 * t + 2 * z1l + 2)
                    scalar.dma_start(
                        out=yv[b0:b0 + SAMPLES_PER_TILE, :, z1l],
                        in_=touts[s][:],
                    ).then_inc(st_sems[s], 16)

    return nc


_NC_CACHE: dict = {}


def _get_nc():
    if "nc" not in _NC_CACHE:
        _NC_CACHE["nc"] = build_nc()
    return _NC_CACHE["nc"]


def kernel(data: np.ndarray, _trace: bool = False):
    data = np.ascontiguousarray(data, dtype=np.float32)
    assert data.shape == (B, 2, 65536), data.shape
    nc = _get_nc()
    in_maps = [{"x": data[i * B_PER_CORE:(i + 1) * B_PER_CORE]}
               for i in range(N_CORES)]
    res = run_bass_kernel_spmd(nc, in_maps, list(range(N_CORES)),
                               trace=_trace)
    out = np.concatenate([res.results[i]["y"] for i in range(N_CORES)], axis=0)
    if _trace:
        return out, res
    return out
